# revision 12
# baseline (speedup 1.0000x reference)
import math
import sys

sys.path.insert(0, "/opt/trn_rl_repo")

import numpy as np

_EXEC_TIME_NS = None


class CFG:
    def __init__(self, n_nodes=100000, n_edges=1600000, hidden=48, layers=4,
                 alpha=0.1, theta=0.5, ncores=8):
        self.N = n_nodes
        self.E = n_edges
        self.F = hidden
        self.L = layers
        self.alpha = alpha
        self.theta = theta
        self.C = ncores
        self.P = 128
        self.EL = 64
        self.NPC = n_nodes // ncores
        self.NT = math.ceil(self.NPC / self.P)
        self.NPAD = self.NT * self.P
        self.NCHUNK = 4
        self.CHUNK = math.ceil(n_nodes / self.NCHUNK)
        assert self.CHUNK < 32768
        self.G = 4
        self.MAXSLOTS = 8
        self.DCH = max(d for d in (512, 448, 384, 256, 128) if self.NPAD % d == 0)


def _build_schedule(cfg, src, dst):
    P, C, NT, NCH = cfg.P, cfg.C, cfg.NT, cfg.NCHUNK
    core = dst // cfg.NPC
    dl = dst - core * cfg.NPC
    tile = dl // P
    lane_dst = dl % P
    chunk = np.minimum(src // cfg.CHUNK, NCH - 1)
    src_rel = src - chunk * cfg.CHUNK

    key = (core * NT + tile) * NCH + chunk
    order = np.argsort(key, kind="stable")
    key_s = key[order]
    src_s = src_rel[order]
    dst_s = lane_dst[order]
    starts = np.searchsorted(key_s, np.arange(C * NT * NCH))
    ends = np.searchsorted(key_s, np.arange(C * NT * NCH) + 1)
    cnt = (ends - starts).reshape(C, NT, NCH)

    n_slots = np.ceil(cnt.max(axis=0) / P).astype(np.int64)

    slot_tile = []
    slot_of = {}
    calls = []
    groups = []
    t0 = 0
    while t0 < NT:
        tiles = list(range(t0, min(t0 + cfg.G, NT)))
        gcalls = []
        for ch in range(NCH):
            run0 = len(slot_tile)
            for t in tiles:
                slot_of[(t, ch)] = len(slot_tile)
                slot_tile.extend([t] * int(n_slots[t, ch]))
            run1 = len(slot_tile)
            sx = run0
            while sx < run1:
                s1 = min(sx + cfg.MAXSLOTS, run1)
                gcalls.append(len(calls))
                calls.append((ch, sx, s1))
                sx = s1
        groups.append((tiles, gcalls))
        t0 += cfg.G
    slot_tile = np.asarray(slot_tile, dtype=np.int64)
    ST = len(slot_tile)

    tile_first = np.full(NT, -1, dtype=np.int64)
    tile_last = np.full(NT, -1, dtype=np.int64)
    for sx, t in enumerate(slot_tile):
        if tile_first[t] < 0:
            tile_first[t] = sx
        tile_last[t] = sx

    idx16 = []
    dstf = []
    for c in range(C):
        flat_idx = np.zeros(ST * P, dtype=np.int64)
        flat_dst = np.full(ST * P, -1.0, dtype=np.float32)
        for t in range(NT):
            for ch in range(NCH):
                ns = int(n_slots[t, ch])
                if ns == 0:
                    continue
                k = (c * NT + t) * NCH + ch
                a, b = starts[k], ends[k]
                n = b - a
                pos = slot_of[(t, ch)] * P
                flat_idx[pos:pos + n] = src_s[a:b]
                flat_dst[pos:pos + n] = dst_s[a:b]
        wrapped = np.zeros((P, ST * 8), dtype=np.int16)
        for (ch, s0, s1) in calls:
            blk = flat_idx[s0 * P:s1 * P]
            w = blk.reshape(-1, 16).T.astype(np.int16)
            wrapped[:, s0 * 8:s1 * 8] = np.tile(w, (8, 1))
        idx16.append(wrapped)
        dstf.append(np.ascontiguousarray(
            flat_dst.reshape(ST, P).T.astype(np.float32)))
    return slot_tile, calls, groups, tile_first, tile_last, ST, idx16, dstf


def _build_bass(cfg, ST, slot_tile, calls, groups, tile_first, tile_last):
    import concourse.bacc as bacc
    import concourse.bass as bass
    import concourse.tile as tile
    from concourse import mybir
    from concourse.masks import make_identity

    P, F, EL, NT, NPC = cfg.P, cfg.F, cfg.EL, cfg.NT, cfg.NPC
    NPAD, DCH = cfg.NPAD, cfg.DCH
    NDC = NPAD // DCH
    assert NPAD % DCH == 0
    f32 = mybir.dt.float32

    betas = [float(np.log(cfg.theta / (l + 1) + 1.0)) for l in range(cfg.L)]

    nc = bacc.Bacc("TRN2", target_bir_lowering=False, debug=False,
                   num_devices=cfg.C)

    xT = nc.dram_tensor("xT", [3, NPAD], f32, kind="ExternalInput")
    idx16 = nc.dram_tensor("idx16", [P, ST * 8], mybir.dt.int16, kind="ExternalInput")
    dstf = nc.dram_tensor("dstf", [P, ST], f32, kind="ExternalInput")
    W0 = nc.dram_tensor("W0", [3, F], f32, kind="ExternalInput")
    b0 = nc.dram_tensor("b0", [F], f32, kind="ExternalInput")
    convW = nc.dram_tensor("convW", [cfg.L, F, F], f32, kind="ExternalInput")
    W1 = nc.dram_tensor("W1", [F, F], f32, kind="ExternalInput")
    b1 = nc.dram_tensor("b1", [F], f32, kind="ExternalInput")
    out_d = nc.dram_tensor("out", [NPC, F], f32, kind="ExternalOutput")

    H = [nc.dram_tensor(f"H{i}", [cfg.N, EL], f32, addr_space="Shared")
         for i in range(2)]
    bounce = [nc.dram_tensor(f"bounce{i}", [NPC, EL], f32) for i in range(2)]
    rg = [list(range(cfg.C))]

    with tile.TileContext(nc) as tc:
        with (
            tc.tile_pool(name="persist", bufs=1) as pp,
            tc.tile_pool(name="gbuf", bufs=6) as gp,
            tc.tile_pool(name="sbatch", bufs=4) as sp,
            tc.tile_pool(name="work", bufs=3) as wp,
            tc.tile_pool(name="psacc", bufs=5, space="PSUM") as pacc,
            tc.tile_pool(name="psmm", bufs=2, space="PSUM") as pmm,
        ):
            idx_sb = pp.tile([P, ST * 8], mybir.dt.int16)
            nc.sync.dma_start(out=idx_sb[:], in_=idx16.ap())
            dst_sb = pp.tile([P, ST], f32)
            nc.sync.dma_start(out=dst_sb[:], in_=dstf.ap())
            W0_sb = pp.tile([3, F], f32)
            nc.sync.dma_start(out=W0_sb[:], in_=W0.ap())
            b0_sb = pp.tile([F, 1], f32)
            nc.sync.dma_start(out=b0_sb[:], in_=b0.ap()[:, None])
            b0s_sb = pp.tile([F, 1], f32)
            nc.vector.tensor_scalar_mul(b0s_sb[:], b0_sb[:], cfg.alpha)
            b1_sb = pp.tile([F, 1], f32)
            nc.sync.dma_start(out=b1_sb[:], in_=b1.ap()[:, None])
            cw_sb = pp.tile([F, cfg.L * F], f32)
            nc.sync.dma_start(
                out=cw_sb[:].rearrange("k (l f) -> k l f", f=F),
                in_=convW.ap().rearrange("l k f -> k l f"))
            ident = pp.tile([P, P], f32)
            make_identity(nc, ident[:])
            iota = pp.tile([P, P], f32)
            nc.gpsimd.iota(iota[:], pattern=[[1, P]], base=0,
                           channel_multiplier=0,
                           allow_small_or_imprecise_dtypes=True)
            Wl_sb = pp.tile([F, cfg.L * F], f32)
            for l in range(cfg.L):
                sl = slice(l * F, (l + 1) * F)
                nc.vector.tensor_scalar_mul(Wl_sb[:, sl], cw_sb[:, sl], betas[l])
                tmp = wp.tile([F, F], f32, tag="wtmp")
                nc.vector.tensor_scalar_mul(tmp[:], ident[:F, :F], 1.0 - betas[l])
                nc.vector.tensor_tensor(out=Wl_sb[:, sl], in0=Wl_sb[:, sl],
                                        in1=tmp[:], op=mybir.AluOpType.add)
            W1_sb = pp.tile([F, F], f32)
            nc.sync.dma_start(out=W1_sb[:], in_=W1.ap())

            x0T = pp.tile([F, NPAD], f32)
            actT = pp.tile([F, NPAD], f32)
            stage = pp.tile([P, NT * EL], f32)
            nc.vector.memset(stage[:], 0.0)

            def write_shard_and_allgather(par):
                nfull = NPC // P
                rem = NPC - nfull * P
                bap = bounce[par].ap()
                nc.sync.dma_start(
                    out=bap[:nfull * P].rearrange("(t p) f -> p t f", p=P),
                    in_=stage[:, :nfull * EL].rearrange("p (t f) -> p t f", f=EL))
                if rem:
                    nc.sync.dma_start(
                        out=bap[nfull * P:NPC],
                        in_=stage[:rem, nfull * EL:(nfull + 1) * EL])
                nc.gpsimd.collective_compute(
                    "AllGather", mybir.AluOpType.bypass, replica_groups=rg,
                    ins=[bap], outs=[H[par].ap()])

            def transpose_back(scale):
                for t in range(NT):
                    ps = pmm.tile([P, F], f32, space="PSUM", tag="pmm",
                                  name=f"ptb{t}")
                    nc.tensor.transpose(out=ps[:], in_=actT[:, t * P:(t + 1) * P],
                                        identity=ident[:F, :F])
                    nc.scalar.activation(
                        out=stage[:, t * EL:t * EL + F], in_=ps[:],
                        func=mybir.ActivationFunctionType.Copy, bias=0.0,
                        scale=scale)

            for c in range(NDC):
                sl = slice(c * DCH, (c + 1) * DCH)
                xb = wp.tile([3, DCH], f32, tag="xb")
                nc.sync.dma_start(out=xb[:], in_=xT.ap()[:, sl])
                ps = pmm.tile([F, DCH], f32, space="PSUM", tag="pmm",
                              name=f"plin{c}")
                nc.tensor.matmul(out=ps[:], lhsT=W0_sb[:], rhs=xb[:],
                                 start=True, stop=True)
                nc.scalar.activation(out=actT[:, sl], in_=ps[:],
                                     func=mybir.ActivationFunctionType.Relu,
                                     bias=b0_sb[:])
                nc.scalar.activation(out=x0T[:, sl], in_=ps[:],
                                     func=mybir.ActivationFunctionType.Relu,
                                     bias=b0s_sb[:], scale=cfg.alpha)
            transpose_back(1.0 - cfg.alpha)
            write_shard_and_allgather(0)

            for l in range(cfg.L):
                Hsrc = H[l % 2]
                Wl = Wl_sb[:, l * F:(l + 1) * F]
                for tiles, gcalls in groups:
                    pst = {t: pacc.tile([EL, P], f32, space="PSUM", tag="pa",
                                        name=f"pa{l}_{t}")
                           for t in tiles}
                    for ci in gcalls:
                        ch, s0, s1 = calls[ci]
                        ns = s1 - s0
                        gb = gp.tile([P, cfg.MAXSLOTS * EL], f32, tag="g")
                        nc.gpsimd.dma_gather(
                            out_ap=gb[:, :ns * EL].rearrange(
                                "p (s f) -> p s f", s=ns, f=EL),
                            in_ap=Hsrc.ap()[ch * cfg.CHUNK:
                                            min((ch + 1) * cfg.CHUNK, cfg.N)],
                            idxs_ap=idx_sb[:, s0 * 8:s1 * 8],
                            num_idxs=ns * P,
                            num_idxs_reg=ns * P,
                            elem_size=EL,
                        )
                        S = sp.tile([P, cfg.MAXSLOTS * P], f32, tag="S")
                        a0 = iota[:]
                        in0 = bass.AP(a0.tensor, a0.offset,
                                      [a0.ap[0], [0, ns], a0.ap[1]])
                        a1 = dst_sb[:, s0:s1]
                        in1 = bass.AP(a1.tensor, a1.offset,
                                      [a1.ap[0], a1.ap[1], [0, P]])
                        nc.vector.tensor_tensor(
                            out=S[:, :ns * P].rearrange("p (s d) -> p s d", d=P),
                            in0=in0, in1=in1, op=mybir.AluOpType.is_equal)
                        for sx in range(s0, s1):
                            t = int(slot_tile[sx])
                            j = sx - s0
                            nc.tensor.matmul(
                                out=pst[t][:],
                                lhsT=gb[:, j * EL:(j + 1) * EL],
                                rhs=S[:, j * P:(j + 1) * P],
                                start=(sx == tile_first[t]),
                                stop=(sx == tile_last[t]))
                    for t in tiles:
                        if tile_first[t] < 0:
                            nc.vector.tensor_copy(
                                out=actT[:, t * P:(t + 1) * P],
                                in_=x0T[:, t * P:(t + 1) * P])
                        else:
                            nc.vector.tensor_tensor(
                                out=actT[:, t * P:(t + 1) * P],
                                in0=pst[t][:F, :],
                                in1=x0T[:, t * P:(t + 1) * P],
                                op=mybir.AluOpType.add)
                for c in range(NDC):
                    sl = slice(c * DCH, (c + 1) * DCH)
                    ps = pmm.tile([F, DCH], f32, space="PSUM", tag="pmm",
                                  name=f"pd{l}_{c}")
                    nc.tensor.matmul(out=ps[:], lhsT=Wl, rhs=actT[:, sl],
                                     start=True, stop=True)
                    nc.scalar.activation(out=actT[:, sl], in_=ps[:],
                                         func=mybir.ActivationFunctionType.Relu)
                if l < cfg.L - 1:
                    transpose_back(1.0 - cfg.alpha)
                    write_shard_and_allgather((l + 1) % 2)

            zT = x0T
            for c in range(NDC):
                sl = slice(c * DCH, (c + 1) * DCH)
                ps = pmm.tile([F, DCH], f32, space="PSUM", tag="pmm",
                              name=f"ph{c}")
                nc.tensor.matmul(out=ps[:], lhsT=W1_sb[:], rhs=actT[:, sl],
                                 start=True, stop=True)
                nc.scalar.activation(out=zT[:, sl], in_=ps[:],
                                     func=mybir.ActivationFunctionType.Identity,
                                     bias=b1_sb[:])
            for t in range(NT):
                ps = pmm.tile([P, F], f32, space="PSUM", tag="pmm",
                              name=f"pz{t}")
                nc.tensor.transpose(out=ps[:], in_=zT[:, t * P:(t + 1) * P],
                                    identity=ident[:F, :F])
                nc.vector.tensor_copy(out=stage[:, t * EL:t * EL + F], in_=ps[:])
            negM = pp.tile([P, NT], f32)
            nc.vector.tensor_reduce(
                out=negM[:],
                in_=stage[:].rearrange("p (t f) -> p t f", f=EL)[:, :, :F],
                axis=mybir.AxisListType.X, op=mybir.AluOpType.max)
            nc.vector.tensor_scalar_mul(negM[:], negM[:], -1.0)
            SS = pp.tile([P, NT], f32)
            for t in range(NT):
                e = wp.tile([P, F], f32, tag="e")
                nc.scalar.activation(out=e[:], in_=stage[:, t * EL:t * EL + F],
                                     func=mybir.ActivationFunctionType.Exp,
                                     bias=negM[:, t:t + 1],
                                     accum_out=SS[:, t:t + 1])
            LNS = pp.tile([P, NT], f32)
            nc.scalar.activation(out=LNS[:], in_=SS[:],
                                 func=mybir.ActivationFunctionType.Ln)
            for t in range(NT):
                nc.vector.tensor_scalar(
                    out=stage[:, t * EL:t * EL + F],
                    in0=stage[:, t * EL:t * EL + F],
                    scalar1=negM[:, t:t + 1], scalar2=LNS[:, t:t + 1],
                    op0=mybir.AluOpType.add, op1=mybir.AluOpType.subtract)
            nfull = NPC // P
            rem = NPC - nfull * P
            nc.sync.dma_start(
                out=out_d.ap()[:nfull * P].rearrange("(t p) f -> p t f", p=P),
                in_=stage[:, :nfull * EL].rearrange(
                    "p (t f) -> p t f", f=EL)[:, :, :F])
            if rem:
                nc.sync.dma_start(
                    out=out_d.ap()[nfull * P:NPC],
                    in_=stage[:rem, nfull * EL:nfull * EL + F])

    nc.compile()
    return nc


def run(inputs, cfg=None, use_sim=False, trace=False):
    global _EXEC_TIME_NS
    if cfg is None:
        cfg = CFG()
    x = np.asarray(inputs["x"], dtype=np.float32)
    edge_index = np.asarray(inputs["edge_index"]).astype(np.int64)
    W0 = np.asarray(inputs["W0"], dtype=np.float32)
    b0 = np.asarray(inputs["b0"], dtype=np.float32)
    convW = np.asarray(inputs["convW"], dtype=np.float32)
    W1 = np.asarray(inputs["W1"], dtype=np.float32)
    b1 = np.asarray(inputs["b1"], dtype=np.float32)

    src, dst = edge_index[0], edge_index[1]
    (slot_tile, calls, groups, tile_first, tile_last,
     ST, idx16, dstf) = _build_schedule(cfg, src, dst)

    nc = _build_bass(cfg, ST, slot_tile, calls, groups, tile_first, tile_last)

    in_maps = []
    for c in range(cfg.C):
        xc = x[c * cfg.NPC:(c + 1) * cfg.NPC]
        xT = np.zeros((3, cfg.NPAD), dtype=np.float32)
        xT[:, :cfg.NPC] = xc.T
        in_maps.append({
            "xT": xT, "idx16": idx16[c], "dstf": dstf[c],
            "W0": W0, "b0": b0, "convW": convW, "W1": W1, "b1": b1,
        })

    if use_sim:
        from concourse.bass_interp import MultiCoreSim
        sim = MultiCoreSim(nc, num_cores=cfg.C, trace=False)
        for c in range(cfg.C):
            for k, v in in_maps[c].items():
                sim.cores[c].tensor(k)[:] = v
        sim.simulate(check_with_hw=False)
        outs = [np.array(sim.cores[c].tensor("out")) for c in range(cfg.C)]
    else:
        from concourse.bass_utils import run_bass_kernel_spmd
        res = run_bass_kernel_spmd(nc, in_maps, core_ids=list(range(cfg.C)),
                                   trace=trace)
        _EXEC_TIME_NS = res.exec_time_ns
        outs = [res.results[c]["out"] for c in range(cfg.C)]

    return np.concatenate(outs, axis=0)[:cfg.N].astype(np.float32)


def kernel(**inputs):
    import os
    trace = bool(os.environ.get("GCN_TRACE"))
    return run(inputs, CFG(), use_sim=False, trace=trace)


# revision 13
# speedup vs baseline: 1.0053x; 1.0053x over previous
import math
import sys

sys.path.insert(0, "/opt/trn_rl_repo")

import numpy as np

_EXEC_TIME_NS = None


class CFG:
    def __init__(self, n_nodes=100000, n_edges=1600000, hidden=48, layers=4,
                 alpha=0.1, theta=0.5, ncores=8):
        self.N = n_nodes
        self.E = n_edges
        self.F = hidden
        self.L = layers
        self.alpha = alpha
        self.theta = theta
        self.C = ncores
        self.P = 128
        self.EL = 64
        self.NPC = n_nodes // ncores
        self.NT = math.ceil(self.NPC / self.P)
        self.NPAD = self.NT * self.P
        self.NCHUNK = 4
        self.CHUNK = math.ceil(n_nodes / self.NCHUNK)
        assert self.CHUNK < 32768
        self.G = 4
        self.MAXSLOTS = 8
        self.DCH = max(d for d in (512, 448, 384, 256, 128) if self.NPAD % d == 0)


def _build_schedule(cfg, src, dst):
    P, C, NT, NCH = cfg.P, cfg.C, cfg.NT, cfg.NCHUNK
    core = dst // cfg.NPC
    dl = dst - core * cfg.NPC
    tile = dl // P
    lane_dst = dl % P
    half = cfg.NPC // 2
    ci_ = src // cfg.NPC
    ii = src - ci_ * cfg.NPC
    vrow = np.where(ii < half, ci_ * half + ii,
                    cfg.N // 2 + ci_ * half + (ii - half))
    chunk = np.minimum(vrow // cfg.CHUNK, NCH - 1)
    src_rel = vrow - chunk * cfg.CHUNK

    key = (core * NT + tile) * NCH + chunk
    order = np.argsort(key, kind="stable")
    key_s = key[order]
    src_s = src_rel[order]
    dst_s = lane_dst[order]
    starts = np.searchsorted(key_s, np.arange(C * NT * NCH))
    ends = np.searchsorted(key_s, np.arange(C * NT * NCH) + 1)
    cnt = (ends - starts).reshape(C, NT, NCH)

    n_slots = np.ceil(cnt.max(axis=0) / P).astype(np.int64)

    slot_tile = []
    slot_of = {}
    calls = []
    groups = []
    t0 = 0
    while t0 < NT:
        tiles = list(range(t0, min(t0 + cfg.G, NT)))
        gcalls = []
        for ch in range(NCH):
            run0 = len(slot_tile)
            for t in tiles:
                slot_of[(t, ch)] = len(slot_tile)
                slot_tile.extend([t] * int(n_slots[t, ch]))
            run1 = len(slot_tile)
            sx = run0
            while sx < run1:
                s1 = min(sx + cfg.MAXSLOTS, run1)
                gcalls.append(len(calls))
                calls.append((ch, sx, s1))
                sx = s1
        groups.append((tiles, gcalls))
        t0 += cfg.G
    slot_tile = np.asarray(slot_tile, dtype=np.int64)
    ST = len(slot_tile)

    tile_first = np.full(NT, -1, dtype=np.int64)
    tile_last = np.full(NT, -1, dtype=np.int64)
    for sx, t in enumerate(slot_tile):
        if tile_first[t] < 0:
            tile_first[t] = sx
        tile_last[t] = sx

    idx16 = []
    dstf = []
    for c in range(C):
        flat_idx = np.zeros(ST * P, dtype=np.int64)
        flat_dst = np.full(ST * P, -1.0, dtype=np.float32)
        for t in range(NT):
            for ch in range(NCH):
                ns = int(n_slots[t, ch])
                if ns == 0:
                    continue
                k = (c * NT + t) * NCH + ch
                a, b = starts[k], ends[k]
                n = b - a
                pos = slot_of[(t, ch)] * P
                flat_idx[pos:pos + n] = src_s[a:b]
                flat_dst[pos:pos + n] = dst_s[a:b]
        wrapped = np.zeros((P, ST * 8), dtype=np.int16)
        for (ch, s0, s1) in calls:
            blk = flat_idx[s0 * P:s1 * P]
            w = blk.reshape(-1, 16).T.astype(np.int16)
            wrapped[:, s0 * 8:s1 * 8] = np.tile(w, (8, 1))
        idx16.append(wrapped)
        dstf.append(np.ascontiguousarray(
            flat_dst.reshape(ST, P).T.astype(np.float32)))
    return slot_tile, calls, groups, tile_first, tile_last, ST, idx16, dstf


def _build_bass(cfg, ST, slot_tile, calls, groups, tile_first, tile_last):
    import concourse.bacc as bacc
    import concourse.bass as bass
    import concourse.tile as tile
    from concourse import mybir
    from concourse.masks import make_identity

    P, F, EL, NT, NPC = cfg.P, cfg.F, cfg.EL, cfg.NT, cfg.NPC
    NPAD, DCH = cfg.NPAD, cfg.DCH
    NDC = NPAD // DCH
    assert NPAD % DCH == 0
    f32 = mybir.dt.float32

    betas = [float(np.log(cfg.theta / (l + 1) + 1.0)) for l in range(cfg.L)]

    nc = bacc.Bacc("TRN2", target_bir_lowering=False, debug=False,
                   num_devices=cfg.C)

    xT = nc.dram_tensor("xT", [3, NPAD], f32, kind="ExternalInput")
    idx16 = nc.dram_tensor("idx16", [P, ST * 8], mybir.dt.int16, kind="ExternalInput")
    dstf = nc.dram_tensor("dstf", [P, ST], f32, kind="ExternalInput")
    W0 = nc.dram_tensor("W0", [3, F], f32, kind="ExternalInput")
    b0 = nc.dram_tensor("b0", [F], f32, kind="ExternalInput")
    convW = nc.dram_tensor("convW", [cfg.L, F, F], f32, kind="ExternalInput")
    W1 = nc.dram_tensor("W1", [F, F], f32, kind="ExternalInput")
    b1 = nc.dram_tensor("b1", [F], f32, kind="ExternalInput")
    out_d = nc.dram_tensor("out", [NPC, F], f32, kind="ExternalOutput")

    NH = cfg.N // 2
    H = [[nc.dram_tensor(f"H{i}a", [NH, EL], f32, addr_space="Shared"),
          nc.dram_tensor(f"H{i}b", [NH, EL], f32, addr_space="Shared")]
         for i in range(2)]
    bounce = [nc.dram_tensor(f"bounce{i}", [NPC, EL], f32) for i in range(2)]
    rg = [list(range(cfg.C))]

    with tile.TileContext(nc) as tc:
        with (
            tc.tile_pool(name="persist", bufs=1) as pp,
            tc.tile_pool(name="gbuf", bufs=6) as gp,
            tc.tile_pool(name="sbatch", bufs=4) as sp,
            tc.tile_pool(name="work", bufs=3) as wp,
            tc.tile_pool(name="psacc", bufs=5, space="PSUM") as pacc,
            tc.tile_pool(name="psmm", bufs=2, space="PSUM") as pmm,
        ):
            idx_sb = pp.tile([P, ST * 8], mybir.dt.int16)
            nc.sync.dma_start(out=idx_sb[:], in_=idx16.ap())
            dst_sb = pp.tile([P, ST], f32)
            nc.sync.dma_start(out=dst_sb[:], in_=dstf.ap())
            W0_sb = pp.tile([3, F], f32)
            nc.sync.dma_start(out=W0_sb[:], in_=W0.ap())
            b0_sb = pp.tile([F, 1], f32)
            nc.sync.dma_start(out=b0_sb[:], in_=b0.ap()[:, None])
            b0s_sb = pp.tile([F, 1], f32)
            nc.vector.tensor_scalar_mul(b0s_sb[:], b0_sb[:], cfg.alpha)
            b1_sb = pp.tile([F, 1], f32)
            nc.sync.dma_start(out=b1_sb[:], in_=b1.ap()[:, None])
            cw_sb = pp.tile([F, cfg.L * F], f32)
            nc.sync.dma_start(
                out=cw_sb[:].rearrange("k (l f) -> k l f", f=F),
                in_=convW.ap().rearrange("l k f -> k l f"))
            ident = pp.tile([P, P], f32)
            make_identity(nc, ident[:])
            iota = pp.tile([P, P], f32)
            nc.gpsimd.iota(iota[:], pattern=[[1, P]], base=0,
                           channel_multiplier=0,
                           allow_small_or_imprecise_dtypes=True)
            Wl_sb = pp.tile([F, cfg.L * F], f32)
            for l in range(cfg.L):
                sl = slice(l * F, (l + 1) * F)
                nc.vector.tensor_scalar_mul(Wl_sb[:, sl], cw_sb[:, sl], betas[l])
                tmp = wp.tile([F, F], f32, tag="wtmp")
                nc.vector.tensor_scalar_mul(tmp[:], ident[:F, :F], 1.0 - betas[l])
                nc.vector.tensor_tensor(out=Wl_sb[:, sl], in0=Wl_sb[:, sl],
                                        in1=tmp[:], op=mybir.AluOpType.add)
            W1_sb = pp.tile([F, F], f32)
            nc.sync.dma_start(out=W1_sb[:], in_=W1.ap())

            x0T = pp.tile([F, NPAD], f32)
            actT = pp.tile([F, NPAD], f32)
            stage = pp.tile([P, NT * EL], f32)
            nc.vector.memset(stage[:], 0.0)

            def write_shard_and_allgather(par):
                nfull = NPC // P
                rem = NPC - nfull * P
                bap = bounce[par].ap()
                nc.sync.dma_start(
                    out=bap[:nfull * P].rearrange("(t p) f -> p t f", p=P),
                    in_=stage[:, :nfull * EL].rearrange("p (t f) -> p t f", f=EL))
                if rem:
                    nc.sync.dma_start(
                        out=bap[nfull * P:NPC],
                        in_=stage[:rem, nfull * EL:(nfull + 1) * EL])
                half = NPC // 2
                nc.gpsimd.collective_compute(
                    "AllGather", mybir.AluOpType.bypass, replica_groups=rg,
                    ins=[bap[:half]], outs=[H[par][0].ap()])
                nc.gpsimd.collective_compute(
                    "AllGather", mybir.AluOpType.bypass, replica_groups=rg,
                    ins=[bap[half:NPC]], outs=[H[par][1].ap()])

            def transpose_back(scale):
                for t in range(NT):
                    ps = pmm.tile([P, F], f32, space="PSUM", tag="pmm",
                                  name=f"ptb{t}")
                    nc.tensor.transpose(out=ps[:], in_=actT[:, t * P:(t + 1) * P],
                                        identity=ident[:F, :F])
                    nc.scalar.activation(
                        out=stage[:, t * EL:t * EL + F], in_=ps[:],
                        func=mybir.ActivationFunctionType.Copy, bias=0.0,
                        scale=scale)

            for c in range(NDC):
                sl = slice(c * DCH, (c + 1) * DCH)
                xb = wp.tile([3, DCH], f32, tag="xb")
                nc.sync.dma_start(out=xb[:], in_=xT.ap()[:, sl])
                ps = pmm.tile([F, DCH], f32, space="PSUM", tag="pmm",
                              name=f"plin{c}")
                nc.tensor.matmul(out=ps[:], lhsT=W0_sb[:], rhs=xb[:],
                                 start=True, stop=True)
                nc.scalar.activation(out=actT[:, sl], in_=ps[:],
                                     func=mybir.ActivationFunctionType.Relu,
                                     bias=b0_sb[:])
                nc.scalar.activation(out=x0T[:, sl], in_=ps[:],
                                     func=mybir.ActivationFunctionType.Relu,
                                     bias=b0s_sb[:], scale=cfg.alpha)
            transpose_back(1.0 - cfg.alpha)
            write_shard_and_allgather(0)

            for l in range(cfg.L):
                Hsrc = H[l % 2]
                NH2 = cfg.N // 2
                Wl = Wl_sb[:, l * F:(l + 1) * F]
                for tiles, gcalls in groups:
                    pst = {t: pacc.tile([EL, P], f32, space="PSUM", tag="pa",
                                        name=f"pa{l}_{t}")
                           for t in tiles}
                    for ci in gcalls:
                        ch, s0, s1 = calls[ci]
                        ns = s1 - s0
                        gb = gp.tile([P, cfg.MAXSLOTS * EL], f32, tag="g")
                        nc.gpsimd.dma_gather(
                            out_ap=gb[:, :ns * EL].rearrange(
                                "p (s f) -> p s f", s=ns, f=EL),
                            in_ap=Hsrc[0 if ch * cfg.CHUNK < NH2 else 1].ap()[
                                ch * cfg.CHUNK - (0 if ch * cfg.CHUNK < NH2
                                                  else NH2):
                                min((ch + 1) * cfg.CHUNK, cfg.N)
                                - (0 if ch * cfg.CHUNK < NH2 else NH2)],
                            idxs_ap=idx_sb[:, s0 * 8:s1 * 8],
                            num_idxs=ns * P,
                            num_idxs_reg=ns * P,
                            elem_size=EL,
                        )
                        S = sp.tile([P, cfg.MAXSLOTS * P], f32, tag="S")
                        a0 = iota[:]
                        in0 = bass.AP(a0.tensor, a0.offset,
                                      [a0.ap[0], [0, ns], a0.ap[1]])
                        a1 = dst_sb[:, s0:s1]
                        in1 = bass.AP(a1.tensor, a1.offset,
                                      [a1.ap[0], a1.ap[1], [0, P]])
                        nc.vector.tensor_tensor(
                            out=S[:, :ns * P].rearrange("p (s d) -> p s d", d=P),
                            in0=in0, in1=in1, op=mybir.AluOpType.is_equal)
                        for sx in range(s0, s1):
                            t = int(slot_tile[sx])
                            j = sx - s0
                            nc.tensor.matmul(
                                out=pst[t][:],
                                lhsT=gb[:, j * EL:(j + 1) * EL],
                                rhs=S[:, j * P:(j + 1) * P],
                                start=(sx == tile_first[t]),
                                stop=(sx == tile_last[t]))
                    for t in tiles:
                        if tile_first[t] < 0:
                            nc.vector.tensor_copy(
                                out=actT[:, t * P:(t + 1) * P],
                                in_=x0T[:, t * P:(t + 1) * P])
                        else:
                            nc.vector.tensor_tensor(
                                out=actT[:, t * P:(t + 1) * P],
                                in0=pst[t][:F, :],
                                in1=x0T[:, t * P:(t + 1) * P],
                                op=mybir.AluOpType.add)
                for c in range(NDC):
                    sl = slice(c * DCH, (c + 1) * DCH)
                    ps = pmm.tile([F, DCH], f32, space="PSUM", tag="pmm",
                                  name=f"pd{l}_{c}")
                    nc.tensor.matmul(out=ps[:], lhsT=Wl, rhs=actT[:, sl],
                                     start=True, stop=True)
                    nc.scalar.activation(out=actT[:, sl], in_=ps[:],
                                         func=mybir.ActivationFunctionType.Relu)
                if l < cfg.L - 1:
                    transpose_back(1.0 - cfg.alpha)
                    write_shard_and_allgather((l + 1) % 2)

            zT = x0T
            for c in range(NDC):
                sl = slice(c * DCH, (c + 1) * DCH)
                ps = pmm.tile([F, DCH], f32, space="PSUM", tag="pmm",
                              name=f"ph{c}")
                nc.tensor.matmul(out=ps[:], lhsT=W1_sb[:], rhs=actT[:, sl],
                                 start=True, stop=True)
                nc.scalar.activation(out=zT[:, sl], in_=ps[:],
                                     func=mybir.ActivationFunctionType.Identity,
                                     bias=b1_sb[:])
            for t in range(NT):
                ps = pmm.tile([P, F], f32, space="PSUM", tag="pmm",
                              name=f"pz{t}")
                nc.tensor.transpose(out=ps[:], in_=zT[:, t * P:(t + 1) * P],
                                    identity=ident[:F, :F])
                nc.vector.tensor_copy(out=stage[:, t * EL:t * EL + F], in_=ps[:])
            negM = pp.tile([P, NT], f32)
            nc.vector.tensor_reduce(
                out=negM[:],
                in_=stage[:].rearrange("p (t f) -> p t f", f=EL)[:, :, :F],
                axis=mybir.AxisListType.X, op=mybir.AluOpType.max)
            nc.vector.tensor_scalar_mul(negM[:], negM[:], -1.0)
            SS = pp.tile([P, NT], f32)
            for t in range(NT):
                e = wp.tile([P, F], f32, tag="e")
                nc.scalar.activation(out=e[:], in_=stage[:, t * EL:t * EL + F],
                                     func=mybir.ActivationFunctionType.Exp,
                                     bias=negM[:, t:t + 1],
                                     accum_out=SS[:, t:t + 1])
            LNS = pp.tile([P, NT], f32)
            nc.scalar.activation(out=LNS[:], in_=SS[:],
                                 func=mybir.ActivationFunctionType.Ln)
            for t in range(NT):
                nc.vector.tensor_scalar(
                    out=stage[:, t * EL:t * EL + F],
                    in0=stage[:, t * EL:t * EL + F],
                    scalar1=negM[:, t:t + 1], scalar2=LNS[:, t:t + 1],
                    op0=mybir.AluOpType.add, op1=mybir.AluOpType.subtract)
            nfull = NPC // P
            rem = NPC - nfull * P
            nc.sync.dma_start(
                out=out_d.ap()[:nfull * P].rearrange("(t p) f -> p t f", p=P),
                in_=stage[:, :nfull * EL].rearrange(
                    "p (t f) -> p t f", f=EL)[:, :, :F])
            if rem:
                nc.sync.dma_start(
                    out=out_d.ap()[nfull * P:NPC],
                    in_=stage[:rem, nfull * EL:nfull * EL + F])

    nc.compile()
    return nc


def run(inputs, cfg=None, use_sim=False, trace=False):
    global _EXEC_TIME_NS
    if cfg is None:
        cfg = CFG()
    x = np.asarray(inputs["x"], dtype=np.float32)
    edge_index = np.asarray(inputs["edge_index"]).astype(np.int64)
    W0 = np.asarray(inputs["W0"], dtype=np.float32)
    b0 = np.asarray(inputs["b0"], dtype=np.float32)
    convW = np.asarray(inputs["convW"], dtype=np.float32)
    W1 = np.asarray(inputs["W1"], dtype=np.float32)
    b1 = np.asarray(inputs["b1"], dtype=np.float32)

    src, dst = edge_index[0], edge_index[1]
    (slot_tile, calls, groups, tile_first, tile_last,
     ST, idx16, dstf) = _build_schedule(cfg, src, dst)

    nc = _build_bass(cfg, ST, slot_tile, calls, groups, tile_first, tile_last)

    in_maps = []
    for c in range(cfg.C):
        xc = x[c * cfg.NPC:(c + 1) * cfg.NPC]
        xT = np.zeros((3, cfg.NPAD), dtype=np.float32)
        xT[:, :cfg.NPC] = xc.T
        in_maps.append({
            "xT": xT, "idx16": idx16[c], "dstf": dstf[c],
            "W0": W0, "b0": b0, "convW": convW, "W1": W1, "b1": b1,
        })

    if use_sim:
        from concourse.bass_interp import MultiCoreSim
        sim = MultiCoreSim(nc, num_cores=cfg.C, trace=False)
        for c in range(cfg.C):
            for k, v in in_maps[c].items():
                sim.cores[c].tensor(k)[:] = v
        sim.simulate(check_with_hw=False)
        outs = [np.array(sim.cores[c].tensor("out")) for c in range(cfg.C)]
    else:
        from concourse.bass_utils import run_bass_kernel_spmd
        res = run_bass_kernel_spmd(nc, in_maps, core_ids=list(range(cfg.C)),
                                   trace=trace)
        _EXEC_TIME_NS = res.exec_time_ns
        outs = [res.results[c]["out"] for c in range(cfg.C)]

    return np.concatenate(outs, axis=0)[:cfg.N].astype(np.float32)


def kernel(**inputs):
    import os
    trace = bool(os.environ.get("GCN_TRACE"))
    return run(inputs, CFG(), use_sim=False, trace=trace)


# revision 14
# speedup vs baseline: 1.0356x; 1.0302x over previous
import math
import sys

sys.path.insert(0, "/opt/trn_rl_repo")

import numpy as np

_EXEC_TIME_NS = None


class CFG:
    def __init__(self, n_nodes=100000, n_edges=1600000, hidden=48, layers=4,
                 alpha=0.1, theta=0.5, ncores=8):
        self.N = n_nodes
        self.E = n_edges
        self.F = hidden
        self.L = layers
        self.alpha = alpha
        self.theta = theta
        self.C = ncores
        self.P = 128
        self.EL = 64
        self.NPC = n_nodes // ncores
        self.NT = math.ceil(self.NPC / self.P)
        self.NPAD = self.NT * self.P
        self.NCHUNK = 4
        self.CHUNK = math.ceil(n_nodes / self.NCHUNK)
        assert self.CHUNK < 32768
        self.G = 4
        self.MAXSLOTS = 8
        self.DCH = max(d for d in (512, 448, 384, 256, 128) if self.NPAD % d == 0)


def _build_schedule(cfg, src, dst):
    P, C, NT, NCH = cfg.P, cfg.C, cfg.NT, cfg.NCHUNK
    core = dst // cfg.NPC
    dl = dst - core * cfg.NPC
    tile = dl // P
    lane_dst = dl % P
    half = cfg.NPC // 2
    ci_ = src // cfg.NPC
    ii = src - ci_ * cfg.NPC
    vrow = np.where(ii < half, ci_ * half + ii,
                    cfg.N // 2 + ci_ * half + (ii - half))
    chunk = np.minimum(vrow // cfg.CHUNK, NCH - 1)
    src_rel = vrow - chunk * cfg.CHUNK

    key = (core * NT + tile) * NCH + chunk
    order = np.argsort(key, kind="stable")
    key_s = key[order]
    src_s = src_rel[order]
    dst_s = lane_dst[order]
    starts = np.searchsorted(key_s, np.arange(C * NT * NCH))
    ends = np.searchsorted(key_s, np.arange(C * NT * NCH) + 1)
    cnt = (ends - starts).reshape(C, NT, NCH)

    n_slots = np.ceil(cnt.max(axis=0) / P).astype(np.int64)

    slot_tile = []
    slot_of = {}
    calls = []
    groups = []
    t0 = 0
    while t0 < NT:
        tiles = list(range(t0, min(t0 + cfg.G, NT)))
        gcalls = []
        for ch in range(NCH):
            run0 = len(slot_tile)
            for t in tiles:
                slot_of[(t, ch)] = len(slot_tile)
                slot_tile.extend([t] * int(n_slots[t, ch]))
            run1 = len(slot_tile)
            sx = run0
            while sx < run1:
                s1 = min(sx + cfg.MAXSLOTS, run1)
                gcalls.append(len(calls))
                calls.append((ch, sx, s1))
                sx = s1
        groups.append((tiles, gcalls))
        t0 += cfg.G
    slot_tile = np.asarray(slot_tile, dtype=np.int64)
    ST = len(slot_tile)

    tile_first = np.full(NT, -1, dtype=np.int64)
    tile_last = np.full(NT, -1, dtype=np.int64)
    for sx, t in enumerate(slot_tile):
        if tile_first[t] < 0:
            tile_first[t] = sx
        tile_last[t] = sx

    idx16 = []
    dstf = []
    for c in range(C):
        flat_idx = np.zeros(ST * P, dtype=np.int64)
        flat_dst = np.full(ST * P, -1.0, dtype=np.float32)
        for t in range(NT):
            for ch in range(NCH):
                ns = int(n_slots[t, ch])
                if ns == 0:
                    continue
                k = (c * NT + t) * NCH + ch
                a, b = starts[k], ends[k]
                n = b - a
                pos = slot_of[(t, ch)] * P
                flat_idx[pos:pos + n] = src_s[a:b]
                flat_dst[pos:pos + n] = dst_s[a:b]
        wrapped = np.zeros((P, ST * 8), dtype=np.int16)
        for (ch, s0, s1) in calls:
            blk = flat_idx[s0 * P:s1 * P]
            w = blk.reshape(-1, 16).T.astype(np.int16)
            wrapped[:, s0 * 8:s1 * 8] = np.tile(w, (8, 1))
        idx16.append(wrapped)
        dstf.append(np.ascontiguousarray(
            flat_dst.reshape(ST, P).T.astype(np.float32)))
    return slot_tile, calls, groups, tile_first, tile_last, ST, idx16, dstf


def _build_bass(cfg, ST, slot_tile, calls, groups, tile_first, tile_last):
    import concourse.bacc as bacc
    import concourse.bass as bass
    import concourse.tile as tile
    from concourse import mybir
    from concourse.masks import make_identity

    P, F, EL, NT, NPC = cfg.P, cfg.F, cfg.EL, cfg.NT, cfg.NPC
    NPAD, DCH = cfg.NPAD, cfg.DCH
    NDC = NPAD // DCH
    assert NPAD % DCH == 0
    f32 = mybir.dt.float32

    betas = [float(np.log(cfg.theta / (l + 1) + 1.0)) for l in range(cfg.L)]

    nc = bacc.Bacc("TRN2", target_bir_lowering=False, debug=False,
                   num_devices=cfg.C)

    xT = nc.dram_tensor("xT", [3, NPAD], f32, kind="ExternalInput")
    idx16 = nc.dram_tensor("idx16", [P, ST * 8], mybir.dt.int16, kind="ExternalInput")
    dstf = nc.dram_tensor("dstf", [P, ST], f32, kind="ExternalInput")
    W0 = nc.dram_tensor("W0", [3, F], f32, kind="ExternalInput")
    b0 = nc.dram_tensor("b0", [F], f32, kind="ExternalInput")
    convW = nc.dram_tensor("convW", [cfg.L, F, F], f32, kind="ExternalInput")
    W1 = nc.dram_tensor("W1", [F, F], f32, kind="ExternalInput")
    b1 = nc.dram_tensor("b1", [F], f32, kind="ExternalInput")
    out_d = nc.dram_tensor("out", [NPC, F], f32, kind="ExternalOutput")

    NH = cfg.N // 2
    H = [[nc.dram_tensor(f"H{i}a", [NH, EL], f32, addr_space="Shared"),
          nc.dram_tensor(f"H{i}b", [NH, EL], f32, addr_space="Shared")]
         for i in range(2)]
    bounce = [nc.dram_tensor(f"bounce{i}", [NPC, EL], f32) for i in range(2)]
    rg = [list(range(cfg.C))]

    with tile.TileContext(nc) as tc:
        with (
            tc.tile_pool(name="persist", bufs=1) as pp,
            tc.tile_pool(name="gbuf", bufs=6) as gp,
            tc.tile_pool(name="sbatch", bufs=4) as sp,
            tc.tile_pool(name="work", bufs=3) as wp,
            tc.tile_pool(name="psacc", bufs=5, space="PSUM") as pacc,
            tc.tile_pool(name="psmm", bufs=2, space="PSUM") as pmm,
        ):
            idx_sb = pp.tile([P, ST * 8], mybir.dt.int16)
            nc.sync.dma_start(out=idx_sb[:], in_=idx16.ap())
            dst_sb = pp.tile([P, ST], f32)
            nc.sync.dma_start(out=dst_sb[:], in_=dstf.ap())
            W0_sb = pp.tile([3, F], f32)
            nc.sync.dma_start(out=W0_sb[:], in_=W0.ap())
            b0_sb = pp.tile([F, 1], f32)
            nc.sync.dma_start(out=b0_sb[:], in_=b0.ap()[:, None])
            b0s_sb = pp.tile([F, 1], f32)
            nc.vector.tensor_scalar_mul(b0s_sb[:], b0_sb[:], cfg.alpha)
            b1_sb = pp.tile([F, 1], f32)
            nc.sync.dma_start(out=b1_sb[:], in_=b1.ap()[:, None])
            cw_sb = pp.tile([F, cfg.L * F], f32)
            nc.sync.dma_start(
                out=cw_sb[:].rearrange("k (l f) -> k l f", f=F),
                in_=convW.ap().rearrange("l k f -> k l f"))
            ident = pp.tile([P, P], f32)
            make_identity(nc, ident[:])
            iota = pp.tile([P, P], f32)
            nc.gpsimd.iota(iota[:], pattern=[[1, P]], base=0,
                           channel_multiplier=0,
                           allow_small_or_imprecise_dtypes=True)
            Wl_sb = pp.tile([F, cfg.L * F], f32)
            for l in range(cfg.L):
                sl = slice(l * F, (l + 1) * F)
                nc.vector.tensor_scalar_mul(Wl_sb[:, sl], cw_sb[:, sl], betas[l])
                tmp = wp.tile([F, F], f32, tag="wtmp")
                nc.vector.tensor_scalar_mul(tmp[:], ident[:F, :F], 1.0 - betas[l])
                nc.vector.tensor_tensor(out=Wl_sb[:, sl], in0=Wl_sb[:, sl],
                                        in1=tmp[:], op=mybir.AluOpType.add)
            W1_sb = pp.tile([F, F], f32)
            nc.sync.dma_start(out=W1_sb[:], in_=W1.ap())

            x0T = pp.tile([F, NPAD], f32)
            actT = pp.tile([F, NPAD], f32)
            stage = pp.tile([P, NT * EL], f32)
            nc.vector.memset(stage[:], 0.0)

            def write_shard_and_allgather(par):
                nfull = NPC // P
                rem = NPC - nfull * P
                bap = bounce[par].ap()
                nc.sync.dma_start(
                    out=bap[:nfull * P].rearrange("(t p) f -> p t f", p=P),
                    in_=stage[:, :nfull * EL].rearrange("p (t f) -> p t f", f=EL))
                if rem:
                    nc.sync.dma_start(
                        out=bap[nfull * P:NPC],
                        in_=stage[:rem, nfull * EL:(nfull + 1) * EL])
                half = NPC // 2
                nc.gpsimd.collective_compute(
                    "AllGather", mybir.AluOpType.bypass, replica_groups=rg,
                    ins=[bap[:half]], outs=[H[par][0].ap()])
                nc.gpsimd.collective_compute(
                    "AllGather", mybir.AluOpType.bypass, replica_groups=rg,
                    ins=[bap[half:NPC]], outs=[H[par][1].ap()])

            def transpose_back(scale):
                for t in range(NT):
                    ps = pmm.tile([P, F], f32, space="PSUM", tag="pmm",
                                  name=f"ptb{t}")
                    nc.tensor.transpose(out=ps[:], in_=actT[:, t * P:(t + 1) * P],
                                        identity=ident[:F, :F])
                    nc.scalar.activation(
                        out=stage[:, t * EL:t * EL + F], in_=ps[:],
                        func=mybir.ActivationFunctionType.Copy, bias=0.0,
                        scale=scale)

            for c in range(NDC):
                sl = slice(c * DCH, (c + 1) * DCH)
                xb = wp.tile([3, DCH], f32, tag="xb")
                nc.sync.dma_start(out=xb[:], in_=xT.ap()[:, sl])
                ps = pmm.tile([F, DCH], f32, space="PSUM", tag="pmm",
                              name=f"plin{c}")
                nc.tensor.matmul(out=ps[:], lhsT=W0_sb[:], rhs=xb[:],
                                 start=True, stop=True)
                nc.scalar.activation(out=actT[:, sl], in_=ps[:],
                                     func=mybir.ActivationFunctionType.Relu,
                                     bias=b0_sb[:])
                nc.scalar.activation(out=x0T[:, sl], in_=ps[:],
                                     func=mybir.ActivationFunctionType.Relu,
                                     bias=b0s_sb[:], scale=cfg.alpha)
            transpose_back(1.0 - cfg.alpha)
            write_shard_and_allgather(0)

            for l in range(cfg.L):
                Hsrc = H[l % 2]
                NH2 = cfg.N // 2
                Wl = Wl_sb[:, l * F:(l + 1) * F]
                for tiles, gcalls in groups:
                    pst = {t: pacc.tile([EL, P], f32, space="PSUM", tag="pa",
                                        name=f"pa{l}_{t}")
                           for t in tiles}
                    for ci in gcalls:
                        ch, s0, s1 = calls[ci]
                        ns = s1 - s0
                        gb = gp.tile([P, cfg.MAXSLOTS * EL], f32, tag="g")
                        nc.gpsimd.dma_gather(
                            out_ap=gb[:, :ns * EL].rearrange(
                                "p (s f) -> p s f", s=ns, f=EL),
                            in_ap=Hsrc[0 if ch * cfg.CHUNK < NH2 else 1].ap()[
                                ch * cfg.CHUNK - (0 if ch * cfg.CHUNK < NH2
                                                  else NH2):
                                min((ch + 1) * cfg.CHUNK, cfg.N)
                                - (0 if ch * cfg.CHUNK < NH2 else NH2)],
                            idxs_ap=idx_sb[:, s0 * 8:s1 * 8],
                            num_idxs=ns * P,
                            num_idxs_reg=ns * P,
                            elem_size=EL,
                        )
                        S = sp.tile([P, cfg.MAXSLOTS * P], f32, tag="S")
                        a0 = iota[:]
                        in0 = bass.AP(a0.tensor, a0.offset,
                                      [a0.ap[0], [0, ns], a0.ap[1]])
                        a1 = dst_sb[:, s0:s1]
                        in1 = bass.AP(a1.tensor, a1.offset,
                                      [a1.ap[0], a1.ap[1], [0, P]])
                        nc.vector.tensor_tensor(
                            out=S[:, :ns * P].rearrange("p (s d) -> p s d", d=P),
                            in0=in0, in1=in1, op=mybir.AluOpType.is_equal)
                        for sx in range(s0, s1):
                            t = int(slot_tile[sx])
                            j = sx - s0
                            nc.tensor.matmul(
                                out=pst[t][:],
                                lhsT=gb[:, j * EL:(j + 1) * EL],
                                rhs=S[:, j * P:(j + 1) * P],
                                start=(sx == tile_first[t]),
                                stop=(sx == tile_last[t]))
                    for t in tiles:
                        if tile_first[t] < 0:
                            nc.vector.tensor_copy(
                                out=actT[:, t * P:(t + 1) * P],
                                in_=x0T[:, t * P:(t + 1) * P])
                        else:
                            nc.vector.tensor_tensor(
                                out=actT[:, t * P:(t + 1) * P],
                                in0=pst[t][:F, :],
                                in1=x0T[:, t * P:(t + 1) * P],
                                op=mybir.AluOpType.add)
                    q0 = tiles[0] * P
                    qw = len(tiles) * P
                    psd = pmm.tile([F, 512], f32, space="PSUM", tag="pmm",
                                   name=f"pd{l}_{tiles[0]}")
                    nc.tensor.matmul(out=psd[:, :qw], lhsT=Wl,
                                     rhs=actT[:, q0:q0 + qw],
                                     start=True, stop=True)
                    nc.scalar.activation(out=actT[:, q0:q0 + qw],
                                         in_=psd[:, :qw],
                                         func=mybir.ActivationFunctionType.Relu)
                    if l < cfg.L - 1:
                        for t in tiles:
                            pstb = pmm.tile([P, F], f32, space="PSUM",
                                            tag="pmm", name=f"ptb{l}_{t}")
                            nc.tensor.transpose(
                                out=pstb[:], in_=actT[:, t * P:(t + 1) * P],
                                identity=ident[:F, :F])
                            nc.scalar.activation(
                                out=stage[:, t * EL:t * EL + F], in_=pstb[:],
                                func=mybir.ActivationFunctionType.Copy,
                                bias=0.0, scale=1.0 - cfg.alpha)
                if l < cfg.L - 1:
                    write_shard_and_allgather((l + 1) % 2)

            zT = x0T
            for c in range(NDC):
                sl = slice(c * DCH, (c + 1) * DCH)
                ps = pmm.tile([F, DCH], f32, space="PSUM", tag="pmm",
                              name=f"ph{c}")
                nc.tensor.matmul(out=ps[:], lhsT=W1_sb[:], rhs=actT[:, sl],
                                 start=True, stop=True)
                nc.scalar.activation(out=zT[:, sl], in_=ps[:],
                                     func=mybir.ActivationFunctionType.Identity,
                                     bias=b1_sb[:])
            for t in range(NT):
                ps = pmm.tile([P, F], f32, space="PSUM", tag="pmm",
                              name=f"pz{t}")
                nc.tensor.transpose(out=ps[:], in_=zT[:, t * P:(t + 1) * P],
                                    identity=ident[:F, :F])
                nc.vector.tensor_copy(out=stage[:, t * EL:t * EL + F], in_=ps[:])
            negM = pp.tile([P, NT], f32)
            nc.vector.tensor_reduce(
                out=negM[:],
                in_=stage[:].rearrange("p (t f) -> p t f", f=EL)[:, :, :F],
                axis=mybir.AxisListType.X, op=mybir.AluOpType.max)
            nc.vector.tensor_scalar_mul(negM[:], negM[:], -1.0)
            SS = pp.tile([P, NT], f32)
            for t in range(NT):
                e = wp.tile([P, F], f32, tag="e")
                nc.scalar.activation(out=e[:], in_=stage[:, t * EL:t * EL + F],
                                     func=mybir.ActivationFunctionType.Exp,
                                     bias=negM[:, t:t + 1],
                                     accum_out=SS[:, t:t + 1])
            LNS = pp.tile([P, NT], f32)
            nc.scalar.activation(out=LNS[:], in_=SS[:],
                                 func=mybir.ActivationFunctionType.Ln)
            for t in range(NT):
                nc.vector.tensor_scalar(
                    out=stage[:, t * EL:t * EL + F],
                    in0=stage[:, t * EL:t * EL + F],
                    scalar1=negM[:, t:t + 1], scalar2=LNS[:, t:t + 1],
                    op0=mybir.AluOpType.add, op1=mybir.AluOpType.subtract)
            nfull = NPC // P
            rem = NPC - nfull * P
            nc.sync.dma_start(
                out=out_d.ap()[:nfull * P].rearrange("(t p) f -> p t f", p=P),
                in_=stage[:, :nfull * EL].rearrange(
                    "p (t f) -> p t f", f=EL)[:, :, :F])
            if rem:
                nc.sync.dma_start(
                    out=out_d.ap()[nfull * P:NPC],
                    in_=stage[:rem, nfull * EL:nfull * EL + F])

    nc.compile()
    return nc


def run(inputs, cfg=None, use_sim=False, trace=False):
    global _EXEC_TIME_NS
    if cfg is None:
        cfg = CFG()
    x = np.asarray(inputs["x"], dtype=np.float32)
    edge_index = np.asarray(inputs["edge_index"]).astype(np.int64)
    W0 = np.asarray(inputs["W0"], dtype=np.float32)
    b0 = np.asarray(inputs["b0"], dtype=np.float32)
    convW = np.asarray(inputs["convW"], dtype=np.float32)
    W1 = np.asarray(inputs["W1"], dtype=np.float32)
    b1 = np.asarray(inputs["b1"], dtype=np.float32)

    src, dst = edge_index[0], edge_index[1]
    (slot_tile, calls, groups, tile_first, tile_last,
     ST, idx16, dstf) = _build_schedule(cfg, src, dst)

    nc = _build_bass(cfg, ST, slot_tile, calls, groups, tile_first, tile_last)

    in_maps = []
    for c in range(cfg.C):
        xc = x[c * cfg.NPC:(c + 1) * cfg.NPC]
        xT = np.zeros((3, cfg.NPAD), dtype=np.float32)
        xT[:, :cfg.NPC] = xc.T
        in_maps.append({
            "xT": xT, "idx16": idx16[c], "dstf": dstf[c],
            "W0": W0, "b0": b0, "convW": convW, "W1": W1, "b1": b1,
        })

    if use_sim:
        from concourse.bass_interp import MultiCoreSim
        sim = MultiCoreSim(nc, num_cores=cfg.C, trace=False)
        for c in range(cfg.C):
            for k, v in in_maps[c].items():
                sim.cores[c].tensor(k)[:] = v
        sim.simulate(check_with_hw=False)
        outs = [np.array(sim.cores[c].tensor("out")) for c in range(cfg.C)]
    else:
        from concourse.bass_utils import run_bass_kernel_spmd
        res = run_bass_kernel_spmd(nc, in_maps, core_ids=list(range(cfg.C)),
                                   trace=trace)
        _EXEC_TIME_NS = res.exec_time_ns
        outs = [res.results[c]["out"] for c in range(cfg.C)]

    return np.concatenate(outs, axis=0)[:cfg.N].astype(np.float32)


def kernel(**inputs):
    import os
    trace = bool(os.environ.get("GCN_TRACE"))
    return run(inputs, CFG(), use_sim=False, trace=trace)


# revision 15
# speedup vs baseline: 1.0471x; 1.0111x over previous
import math
import sys

sys.path.insert(0, "/opt/trn_rl_repo")

import numpy as np

_EXEC_TIME_NS = None


class CFG:
    def __init__(self, n_nodes=100000, n_edges=1600000, hidden=48, layers=4,
                 alpha=0.1, theta=0.5, ncores=8):
        self.N = n_nodes
        self.E = n_edges
        self.F = hidden
        self.L = layers
        self.alpha = alpha
        self.theta = theta
        self.C = ncores
        self.P = 128
        self.EL = 64
        self.NPC = n_nodes // ncores
        self.NT = math.ceil(self.NPC / self.P)
        self.NPAD = self.NT * self.P
        self.NCHUNK = 4
        self.CHUNK = math.ceil(n_nodes / self.NCHUNK)
        assert self.CHUNK < 32768
        self.G = 4
        self.MAXSLOTS = 8
        self.DCH = max(d for d in (512, 448, 384, 256, 128) if self.NPAD % d == 0)


def _build_schedule(cfg, src, dst):
    P, C, NT, NCH = cfg.P, cfg.C, cfg.NT, cfg.NCHUNK
    core = dst // cfg.NPC
    dl = dst - core * cfg.NPC
    tile = dl // P
    lane_dst = dl % P
    half = cfg.NPC // 2
    ci_ = src // cfg.NPC
    ii = src - ci_ * cfg.NPC
    vrow = np.where(ii < half, ci_ * half + ii,
                    cfg.N // 2 + ci_ * half + (ii - half))
    chunk = np.minimum(vrow // cfg.CHUNK, NCH - 1)
    src_rel = vrow - chunk * cfg.CHUNK

    key = (core * NT + tile) * NCH + chunk
    order = np.argsort(key, kind="stable")
    key_s = key[order]
    src_s = src_rel[order]
    dst_s = lane_dst[order]
    starts = np.searchsorted(key_s, np.arange(C * NT * NCH))
    ends = np.searchsorted(key_s, np.arange(C * NT * NCH) + 1)
    cnt = (ends - starts).reshape(C, NT, NCH)

    n_slots = np.ceil(cnt.max(axis=0) / P).astype(np.int64)

    slot_tile = []
    slot_of = {}
    calls = []
    groups = []
    t0 = 0
    while t0 < NT:
        tiles = list(range(t0, min(t0 + cfg.G, NT)))
        gcalls = []
        for ch in range(NCH):
            run0 = len(slot_tile)
            for t in tiles:
                slot_of[(t, ch)] = len(slot_tile)
                slot_tile.extend([t] * int(n_slots[t, ch]))
            run1 = len(slot_tile)
            sx = run0
            while sx < run1:
                s1 = min(sx + cfg.MAXSLOTS, run1)
                gcalls.append(len(calls))
                calls.append((ch, sx, s1))
                sx = s1
        groups.append((tiles, gcalls))
        t0 += cfg.G
    slot_tile = np.asarray(slot_tile, dtype=np.int64)
    ST = len(slot_tile)

    tile_first = np.full(NT, -1, dtype=np.int64)
    tile_last = np.full(NT, -1, dtype=np.int64)
    for sx, t in enumerate(slot_tile):
        if tile_first[t] < 0:
            tile_first[t] = sx
        tile_last[t] = sx

    idx16 = []
    dstf = []
    for c in range(C):
        flat_idx = np.zeros(ST * P, dtype=np.int64)
        flat_dst = np.full(ST * P, -1.0, dtype=np.float32)
        for t in range(NT):
            for ch in range(NCH):
                ns = int(n_slots[t, ch])
                if ns == 0:
                    continue
                k = (c * NT + t) * NCH + ch
                a, b = starts[k], ends[k]
                n = b - a
                pos = slot_of[(t, ch)] * P
                flat_idx[pos:pos + n] = src_s[a:b]
                flat_dst[pos:pos + n] = dst_s[a:b]
        wrapped = np.zeros((P, ST * 8), dtype=np.int16)
        for (ch, s0, s1) in calls:
            blk = flat_idx[s0 * P:s1 * P]
            w = blk.reshape(-1, 16).T.astype(np.int16)
            wrapped[:, s0 * 8:s1 * 8] = np.tile(w, (8, 1))
        idx16.append(wrapped)
        dstf.append(np.ascontiguousarray(
            flat_dst.reshape(ST, P).T.astype(np.float32)))
    return slot_tile, calls, groups, tile_first, tile_last, ST, idx16, dstf


def _build_bass(cfg, ST, slot_tile, calls, groups, tile_first, tile_last):
    import concourse.bacc as bacc
    import concourse.bass as bass
    import concourse.tile as tile
    from concourse import mybir
    from concourse.masks import make_identity

    P, F, EL, NT, NPC = cfg.P, cfg.F, cfg.EL, cfg.NT, cfg.NPC
    NPAD, DCH = cfg.NPAD, cfg.DCH
    NDC = NPAD // DCH
    assert NPAD % DCH == 0
    f32 = mybir.dt.float32

    betas = [float(np.log(cfg.theta / (l + 1) + 1.0)) for l in range(cfg.L)]

    nc = bacc.Bacc("TRN2", target_bir_lowering=False, debug=False,
                   num_devices=cfg.C)

    xT = nc.dram_tensor("xT", [3, NPAD], f32, kind="ExternalInput")
    idx16 = nc.dram_tensor("idx16", [P, ST * 8], mybir.dt.int16, kind="ExternalInput")
    dstf = nc.dram_tensor("dstf", [P, ST], f32, kind="ExternalInput")
    W0 = nc.dram_tensor("W0", [3, F], f32, kind="ExternalInput")
    b0 = nc.dram_tensor("b0", [F], f32, kind="ExternalInput")
    convW = nc.dram_tensor("convW", [cfg.L, F, F], f32, kind="ExternalInput")
    W1 = nc.dram_tensor("W1", [F, F], f32, kind="ExternalInput")
    b1 = nc.dram_tensor("b1", [F], f32, kind="ExternalInput")
    out_d = nc.dram_tensor("out", [NPC, F], f32, kind="ExternalOutput")

    NH = cfg.N // 2
    H = [[nc.dram_tensor(f"H{i}a", [NH, EL], f32, addr_space="Shared"),
          nc.dram_tensor(f"H{i}b", [NH, EL], f32, addr_space="Shared")]
         for i in range(2)]
    bounce = [nc.dram_tensor(f"bounce{i}", [NPC, EL], f32) for i in range(2)]
    rg = [list(range(cfg.C))]

    with tile.TileContext(nc) as tc:
        with (
            tc.tile_pool(name="persist", bufs=1) as pp,
            tc.tile_pool(name="gbuf", bufs=6) as gp,
            tc.tile_pool(name="sbatch", bufs=4) as sp,
            tc.tile_pool(name="work", bufs=3) as wp,
            tc.tile_pool(name="psacc", bufs=5, space="PSUM") as pacc,
            tc.tile_pool(name="psmm", bufs=2, space="PSUM") as pmm,
        ):
            idx_sb = pp.tile([P, ST * 8], mybir.dt.int16)
            nc.sync.dma_start(out=idx_sb[:], in_=idx16.ap())
            dst_sb = pp.tile([P, ST], f32)
            nc.sync.dma_start(out=dst_sb[:], in_=dstf.ap())
            W0_sb = pp.tile([3, F], f32)
            nc.sync.dma_start(out=W0_sb[:], in_=W0.ap())
            b0_sb = pp.tile([F, 1], f32)
            nc.sync.dma_start(out=b0_sb[:], in_=b0.ap()[:, None])
            b0s_sb = pp.tile([F, 1], f32)
            nc.vector.tensor_scalar_mul(b0s_sb[:], b0_sb[:], cfg.alpha)
            b1_sb = pp.tile([F, 1], f32)
            nc.sync.dma_start(out=b1_sb[:], in_=b1.ap()[:, None])
            cw_sb = pp.tile([F, cfg.L * F], f32)
            nc.sync.dma_start(
                out=cw_sb[:].rearrange("k (l f) -> k l f", f=F),
                in_=convW.ap().rearrange("l k f -> k l f"))
            ident = pp.tile([P, P], f32)
            make_identity(nc, ident[:])
            iota = pp.tile([P, P], f32)
            nc.gpsimd.iota(iota[:], pattern=[[1, P]], base=0,
                           channel_multiplier=0,
                           allow_small_or_imprecise_dtypes=True)
            Wl_sb = pp.tile([F, cfg.L * F], f32)
            for l in range(cfg.L):
                sl = slice(l * F, (l + 1) * F)
                nc.vector.tensor_scalar_mul(Wl_sb[:, sl], cw_sb[:, sl], betas[l])
                tmp = wp.tile([F, F], f32, tag="wtmp")
                nc.vector.tensor_scalar_mul(tmp[:], ident[:F, :F], 1.0 - betas[l])
                nc.vector.tensor_tensor(out=Wl_sb[:, sl], in0=Wl_sb[:, sl],
                                        in1=tmp[:], op=mybir.AluOpType.add)
            W1_sb = pp.tile([F, F], f32)
            nc.sync.dma_start(out=W1_sb[:], in_=W1.ap())

            x0T = pp.tile([F, NPAD], f32)
            actT = pp.tile([F, NPAD], f32)
            stage = pp.tile([P, NT * EL], f32)
            nc.vector.memset(stage[:], 0.0)

            def write_shard_and_allgather(par):
                nfull = NPC // P
                rem = NPC - nfull * P
                bap = bounce[par].ap()
                nc.sync.dma_start(
                    out=bap[:nfull * P].rearrange("(t p) f -> p t f", p=P),
                    in_=stage[:, :nfull * EL].rearrange("p (t f) -> p t f", f=EL))
                if rem:
                    nc.sync.dma_start(
                        out=bap[nfull * P:NPC],
                        in_=stage[:rem, nfull * EL:(nfull + 1) * EL])
                half = NPC // 2
                nc.gpsimd.collective_compute(
                    "AllGather", mybir.AluOpType.bypass, replica_groups=rg,
                    ins=[bap[:half]], outs=[H[par][0].ap()])
                nc.gpsimd.collective_compute(
                    "AllGather", mybir.AluOpType.bypass, replica_groups=rg,
                    ins=[bap[half:NPC]], outs=[H[par][1].ap()])

            def transpose_back(scale):
                for t in range(NT):
                    ps = pmm.tile([P, F], f32, space="PSUM", tag="pmm",
                                  name=f"ptb{t}")
                    nc.tensor.transpose(out=ps[:], in_=actT[:, t * P:(t + 1) * P],
                                        identity=ident[:F, :F])
                    nc.scalar.activation(
                        out=stage[:, t * EL:t * EL + F], in_=ps[:],
                        func=mybir.ActivationFunctionType.Copy, bias=0.0,
                        scale=scale)

            for c in range(NDC):
                sl = slice(c * DCH, (c + 1) * DCH)
                xb = wp.tile([3, DCH], f32, tag="xb")
                nc.sync.dma_start(out=xb[:], in_=xT.ap()[:, sl])
                ps = pmm.tile([F, DCH], f32, space="PSUM", tag="pmm",
                              name=f"plin{c}")
                nc.tensor.matmul(out=ps[:], lhsT=W0_sb[:], rhs=xb[:],
                                 start=True, stop=True)
                nc.scalar.activation(out=actT[:, sl], in_=ps[:],
                                     func=mybir.ActivationFunctionType.Relu,
                                     bias=b0_sb[:])
                nc.scalar.activation(out=x0T[:, sl], in_=ps[:],
                                     func=mybir.ActivationFunctionType.Relu,
                                     bias=b0s_sb[:], scale=cfg.alpha)
            transpose_back(1.0 - cfg.alpha)
            write_shard_and_allgather(0)

            for l in range(cfg.L):
                Hsrc = H[l % 2]
                NH2 = cfg.N // 2
                Wl = Wl_sb[:, l * F:(l + 1) * F]
                for tiles, gcalls in groups:
                    pst = {t: pacc.tile([EL, P], f32, space="PSUM", tag="pa",
                                        name=f"pa{l}_{t}")
                           for t in tiles}
                    for ci in gcalls:
                        ch, s0, s1 = calls[ci]
                        ns = s1 - s0
                        gb = gp.tile([P, cfg.MAXSLOTS * EL], f32, tag="g")
                        nc.gpsimd.dma_gather(
                            out_ap=gb[:, :ns * EL].rearrange(
                                "p (s f) -> p s f", s=ns, f=EL),
                            in_ap=Hsrc[0 if ch * cfg.CHUNK < NH2 else 1].ap()[
                                ch * cfg.CHUNK - (0 if ch * cfg.CHUNK < NH2
                                                  else NH2):
                                min((ch + 1) * cfg.CHUNK, cfg.N)
                                - (0 if ch * cfg.CHUNK < NH2 else NH2)],
                            idxs_ap=idx_sb[:, s0 * 8:s1 * 8],
                            num_idxs=ns * P,
                            num_idxs_reg=ns * P,
                            elem_size=EL,
                        )
                        S = sp.tile([P, cfg.MAXSLOTS * P], f32, tag="S")
                        a0 = iota[:]
                        in0 = bass.AP(a0.tensor, a0.offset,
                                      [a0.ap[0], [0, ns], a0.ap[1]])
                        a1 = dst_sb[:, s0:s1]
                        in1 = bass.AP(a1.tensor, a1.offset,
                                      [a1.ap[0], a1.ap[1], [0, P]])
                        nc.vector.tensor_tensor(
                            out=S[:, :ns * P].rearrange("p (s d) -> p s d", d=P),
                            in0=in0, in1=in1, op=mybir.AluOpType.is_equal)
                        for sx in range(s0, s1):
                            t = int(slot_tile[sx])
                            j = sx - s0
                            nc.tensor.matmul(
                                out=pst[t][:],
                                lhsT=gb[:, j * EL:(j + 1) * EL],
                                rhs=S[:, j * P:(j + 1) * P],
                                start=(sx == tile_first[t]),
                                stop=(sx == tile_last[t]))
                    for t in tiles:
                        if tile_first[t] < 0:
                            nc.vector.tensor_copy(
                                out=actT[:, t * P:(t + 1) * P],
                                in_=x0T[:, t * P:(t + 1) * P])
                        else:
                            nc.vector.tensor_tensor(
                                out=actT[:, t * P:(t + 1) * P],
                                in0=pst[t][:F, :],
                                in1=x0T[:, t * P:(t + 1) * P],
                                op=mybir.AluOpType.add)
                    q0 = tiles[0] * P
                    qw = len(tiles) * P
                    psd = pmm.tile([F, 512], f32, space="PSUM", tag="pmm",
                                   name=f"pd{l}_{tiles[0]}")
                    nc.tensor.matmul(out=psd[:, :qw], lhsT=Wl,
                                     rhs=actT[:, q0:q0 + qw],
                                     start=True, stop=True)
                    nc.scalar.activation(out=actT[:, q0:q0 + qw],
                                         in_=psd[:, :qw],
                                         func=mybir.ActivationFunctionType.Relu)
                    if l < cfg.L - 1:
                        for t in tiles:
                            pstb = pmm.tile([P, F], f32, space="PSUM",
                                            tag="pmm", name=f"ptb{l}_{t}")
                            nc.tensor.transpose(
                                out=pstb[:], in_=actT[:, t * P:(t + 1) * P],
                                identity=ident[:F, :F])
                            nc.scalar.activation(
                                out=stage[:, t * EL:t * EL + F], in_=pstb[:],
                                func=mybir.ActivationFunctionType.Copy,
                                bias=0.0, scale=1.0 - cfg.alpha)
                    else:
                        psz = pmm.tile([F, 512], f32, space="PSUM",
                                       tag="pmm", name=f"pz_{tiles[0]}")
                        nc.tensor.matmul(out=psz[:, :qw], lhsT=W1_sb[:],
                                         rhs=actT[:, q0:q0 + qw],
                                         start=True, stop=True)
                        nc.scalar.activation(
                            out=x0T[:, q0:q0 + qw], in_=psz[:, :qw],
                            func=mybir.ActivationFunctionType.Identity,
                            bias=b1_sb[:])
                        for t in tiles:
                            pzt = pmm.tile([P, F], f32, space="PSUM",
                                           tag="pmm", name=f"pzt{t}")
                            nc.tensor.transpose(
                                out=pzt[:], in_=x0T[:, t * P:(t + 1) * P],
                                identity=ident[:F, :F])
                            nc.vector.tensor_copy(
                                out=stage[:, t * EL:t * EL + F], in_=pzt[:])
                if l < cfg.L - 1:
                    write_shard_and_allgather((l + 1) % 2)

            negM = pp.tile([P, NT], f32)
            nc.vector.tensor_reduce(
                out=negM[:],
                in_=stage[:].rearrange("p (t f) -> p t f", f=EL)[:, :, :F],
                axis=mybir.AxisListType.X, op=mybir.AluOpType.max)
            nc.vector.tensor_scalar_mul(negM[:], negM[:], -1.0)
            SS = pp.tile([P, NT], f32)
            for t in range(NT):
                e = wp.tile([P, F], f32, tag="e")
                nc.scalar.activation(out=e[:], in_=stage[:, t * EL:t * EL + F],
                                     func=mybir.ActivationFunctionType.Exp,
                                     bias=negM[:, t:t + 1],
                                     accum_out=SS[:, t:t + 1])
            LNS = pp.tile([P, NT], f32)
            nc.scalar.activation(out=LNS[:], in_=SS[:],
                                 func=mybir.ActivationFunctionType.Ln)
            for t in range(NT):
                nc.vector.tensor_scalar(
                    out=stage[:, t * EL:t * EL + F],
                    in0=stage[:, t * EL:t * EL + F],
                    scalar1=negM[:, t:t + 1], scalar2=LNS[:, t:t + 1],
                    op0=mybir.AluOpType.add, op1=mybir.AluOpType.subtract)
            nfull = NPC // P
            rem = NPC - nfull * P
            nc.sync.dma_start(
                out=out_d.ap()[:nfull * P].rearrange("(t p) f -> p t f", p=P),
                in_=stage[:, :nfull * EL].rearrange(
                    "p (t f) -> p t f", f=EL)[:, :, :F])
            if rem:
                nc.sync.dma_start(
                    out=out_d.ap()[nfull * P:NPC],
                    in_=stage[:rem, nfull * EL:nfull * EL + F])

    nc.compile()
    return nc


def run(inputs, cfg=None, use_sim=False, trace=False):
    global _EXEC_TIME_NS
    if cfg is None:
        cfg = CFG()
    x = np.asarray(inputs["x"], dtype=np.float32)
    edge_index = np.asarray(inputs["edge_index"]).astype(np.int64)
    W0 = np.asarray(inputs["W0"], dtype=np.float32)
    b0 = np.asarray(inputs["b0"], dtype=np.float32)
    convW = np.asarray(inputs["convW"], dtype=np.float32)
    W1 = np.asarray(inputs["W1"], dtype=np.float32)
    b1 = np.asarray(inputs["b1"], dtype=np.float32)

    src, dst = edge_index[0], edge_index[1]
    (slot_tile, calls, groups, tile_first, tile_last,
     ST, idx16, dstf) = _build_schedule(cfg, src, dst)

    nc = _build_bass(cfg, ST, slot_tile, calls, groups, tile_first, tile_last)

    in_maps = []
    for c in range(cfg.C):
        xc = x[c * cfg.NPC:(c + 1) * cfg.NPC]
        xT = np.zeros((3, cfg.NPAD), dtype=np.float32)
        xT[:, :cfg.NPC] = xc.T
        in_maps.append({
            "xT": xT, "idx16": idx16[c], "dstf": dstf[c],
            "W0": W0, "b0": b0, "convW": convW, "W1": W1, "b1": b1,
        })

    if use_sim:
        from concourse.bass_interp import MultiCoreSim
        sim = MultiCoreSim(nc, num_cores=cfg.C, trace=False)
        for c in range(cfg.C):
            for k, v in in_maps[c].items():
                sim.cores[c].tensor(k)[:] = v
        sim.simulate(check_with_hw=False)
        outs = [np.array(sim.cores[c].tensor("out")) for c in range(cfg.C)]
    else:
        from concourse.bass_utils import run_bass_kernel_spmd
        res = run_bass_kernel_spmd(nc, in_maps, core_ids=list(range(cfg.C)),
                                   trace=trace)
        _EXEC_TIME_NS = res.exec_time_ns
        outs = [res.results[c]["out"] for c in range(cfg.C)]

    return np.concatenate(outs, axis=0)[:cfg.N].astype(np.float32)


def kernel(**inputs):
    import os
    trace = bool(os.environ.get("GCN_TRACE"))
    return run(inputs, CFG(), use_sim=False, trace=trace)
